# revision 1
# baseline (speedup 1.0000x reference)
"""Trainium2 Bass kernel for DeformableAttentionV2 (nn_DeformableAttentionV2_74904229642680).

Sharding: queries split across 8 cores (2500 each, padded to 2560 = 20 tiles x
128); value feature maps + linear weights replicated on every core.

Per core, per tile of 128 queries (query index on SBUF partitions):
  1. PE matmul: sampling offsets off[q, 36] = Q @ W_off  (Q^T staged by host).
  2. DVE: padded pixel coords = off + refpx (host folds 128*ref + b_off + 3.5),
     exact floor via the 2^23 magic trick, clamp into the zero-padded border,
     fractional bilinear weights, int32 row-pair gather indices.
  3. One indirect DMA gather: 36 row-pairs x 512 contiguous fp32 per query
     (2KB descriptors) from the zero-padded value table [2*136*136, 256].
  4. dots d[q, 72] = <query, corner>: scalar_tensor_tensor with accum_out,
     split across DVE and GPSIMD.
  5. logits = 16 * sum(w * d) per point; softmax over 18 points (exp on ACT).
  6. out[q, 256] = (sum_j (exp*w)_j * corner_j) / sum_exp: stt FMA chains on
     DVE + GPSIMD.

Zero padding of the value maps implements grid_sample's padding_mode='zeros'
and corner-validity masking exactly: out-of-range corners read zeros, so
their bilinear weights contribute nothing.
"""

import numpy as np
from contextlib import ExitStack

import concourse.bass as bass
import concourse.bacc as bacc_mod
import concourse.mybir as mybir
from concourse.bass import IndirectOffsetOnAxis
from concourse.tile import TileContext

# ---- hardcoded problem geometry ----
NQ = 20000
C = 256
NLEV = 2
NPT = 9                        # points per level
NLP = NLEV * NPT               # 18
H = W = 128
PAD = 4
HP = WP = H + 2 * PAD          # 136
LVL_ROWS = HP * WP             # 18496
VP_ROWS = NLEV * LVL_ROWS      # 36992
N_CORES = 8
NQ_CORE = 2560                 # padded per-core queries (20 tiles x 128)
NQ_REAL = 2500
P = 128
NOFF = NLP * 2                 # 36 offset scalars per query (l, p, xy)
NPAIR = NLP * 2                # 36 gathered row-pairs per query (l, p, dy)
NCORN = NPAIR * 2              # 72 corners (l, p, dy, dx)
MAGIC = float(np.float32(2.0 ** 23))
NT_FULL = NQ_CORE // P         # 20
CONST_FREE = 2 * NOFF + NT_FULL * NOFF  # 792

F32 = mybir.dt.float32
I32 = mybir.dt.int32
AL = mybir.AluOpType
AX = mybir.AxisListType


def build_nc(n_tiles=NQ_CORE // P, stop_after=None):
    """Build the per-core Bass program. Same program runs SPMD on all cores."""
    nc = bacc_mod.Bacc()

    q_in = nc.declare_dram_parameter("q", [NQ_CORE, C], F32, isOutput=False)
    qt_in = nc.declare_dram_parameter("qt", [2 * P, NQ_CORE], F32, isOutput=False)
    consts_in = nc.declare_dram_parameter("consts", [P, CONST_FREE], F32,
                                          isOutput=False)
    vp_in = nc.declare_dram_parameter("vp", [VP_ROWS, 2 * C], F32, isOutput=False)
    out_ext = nc.declare_dram_parameter("out", [NQ_CORE, C], F32, isOutput=True)

    ctx = ExitStack()
    with ctx:
        tc = ctx.enter_context(TileContext(nc))

        const_pool = ctx.enter_context(tc.tile_pool(name="const", bufs=1))
        io_pool = ctx.enter_context(tc.tile_pool(name="io", bufs=2))
        g_pool = ctx.enter_context(tc.tile_pool(name="g", bufs=2))
        sm_pool = ctx.enter_context(tc.tile_pool(name="sm", bufs=2))
        ps_pool = ctx.enter_context(tc.tile_pool(name="ps", bufs=2, space="PSUM"))
        one_pool = ctx.enter_context(tc.tile_pool(name="one", bufs=1))

        # resident constants: W_off | refpx | Q^T packed in one tensor -> 1 DMA
        consts_sb = const_pool.tile([P, CONST_FREE], F32, name="consts_sb")
        nc.sync.dma_start(out=consts_sb[:], in_=consts_in[:, :])
        w_sb = consts_sb[:, 0:2 * NOFF].rearrange("p (k j) -> p k j", k=2)
        refpx_full = consts_sb[:, 2 * NOFF:].rearrange("p (t j) -> p t j", j=NOFF)

        for t in range(n_tiles):
            # ---- tile input loads ----
            q_sb = io_pool.tile([P, C], F32, name="q_sb")
            nc.sync.dma_start(out=q_sb[:], in_=q_in[bass.ts(t, P), :])
            qt_sb = io_pool.tile([P, 2, P], F32, name="qt_sb")
            nc.sync.dma_start(
                out=qt_sb[:],
                in_=qt_in[:, :].rearrange("(k c) q -> c k q", k=2)[:, :, bass.ts(t, P)],
            )

            # ---- offsets matmul: off[q, 36] = Q @ W_off ----
            off_ps = ps_pool.tile([P, NOFF], F32, space="PSUM", name="off_ps")
            for k in range(2):
                nc.tensor.matmul(
                    out=off_ps[:],
                    lhsT=qt_sb[:, k, :],
                    rhs=w_sb[:, k, :],
                    start=(k == 0),
                    stop=(k == 1),
                )

            # ---- coords / floor / weights / indices ----
            coord = sm_pool.tile([P, NOFF], F32, name="coord")
            nc.vector.tensor_tensor(coord[:], off_ps[:], refpx_full[:, t, :], AL.add)

            rnd = sm_pool.tile([P, NOFF], F32, name="rnd")
            nc.vector.tensor_scalar(rnd[:], coord[:], MAGIC, MAGIC,
                                    AL.add, AL.subtract)
            gt = sm_pool.tile([P, NOFF], F32, name="gt")
            nc.vector.tensor_tensor(gt[:], rnd[:], coord[:], AL.is_gt)
            fl = sm_pool.tile([P, NOFF], F32, name="fl")
            nc.vector.tensor_tensor(fl[:], rnd[:], gt[:], AL.subtract)
            nc.vector.tensor_scalar(fl[:], fl[:], 0.0, 134.0, AL.max, AL.min)
            frac = sm_pool.tile([P, NOFF], F32, name="frac")
            nc.vector.tensor_tensor(frac[:], coord[:], fl[:], AL.subtract)

            # corner weights w[q, (l,p,dy,dx)] = wy[dy] * wx[dx]
            wx01 = sm_pool.tile([P, NLP, 2], F32, name="wx01")
            wy01 = sm_pool.tile([P, NLP, 2], F32, name="wy01")
            for arr, xy in ((wx01, 0), (wy01, 1)):
                fr = frac[:, xy:NOFF:2]
                nc.vector.tensor_scalar(arr[:, :, 0], fr, -1.0, 1.0, AL.mult, AL.add)
                nc.vector.tensor_copy(arr[:, :, 1], fr)
            wcorn = sm_pool.tile([P, NCORN], F32, name="wcorn")
            nc.vector.tensor_tensor(
                wcorn[:].rearrange("p (k y x) -> p k y x", y=2, x=2),
                wy01[:, :, :, None].broadcast_to([P, NLP, 2, 2]),
                wx01[:, :, None, :].broadcast_to([P, NLP, 2, 2]),
                AL.mult,
            )

            # gather indices: idx[q, (l,p)] = y0*136 + x0 + l*18496
            basef = sm_pool.tile([P, NLP], F32, name="basef")
            nc.vector.tensor_scalar(basef[:], fl[:, 1:NOFF:2], 136.0, None, AL.mult)
            nc.vector.tensor_tensor(basef[:], basef[:], fl[:, 0:NOFF:2], AL.add)
            nc.vector.tensor_scalar(basef[:, NPT:], basef[:, NPT:],
                                    float(LVL_ROWS), None, AL.add)
            idxi = sm_pool.tile([P, NLP], I32, name="idxi")
            nc.vector.tensor_copy(idxi[:], basef[:])

            # ---- gathers: per point, one [P,1] indirect fetch of 1024 fp32
            # from the pair-interleaved table (vp2 row (l,y,x) holds
            # [v[y,x,:], v[y+1,x,:]]); reading 1024 elems from row
            # (l, y0, x0) yields all 4 bilinear corners as [dx, dy, C].
            g_sb = g_pool.tile([P, NLP, 1024], F32, name="g_sb")
            for pt in range(NLP):
                nc.gpsimd.indirect_dma_start(
                    out=g_sb[:, pt, :],
                    out_offset=None,
                    in_=vp_in[:, :],
                    in_offset=IndirectOffsetOnAxis(ap=idxi[:, pt:pt + 1], axis=0),
                )

            if stop_after == "gather":
                nc.sync.dma_start(out=out_ext[bass.ts(t, P), :],
                                  in_=g_sb[:, 0, 0:C])
                continue

            # ---- kv per point: bilinear combine of the 4 corners ----
            kv = one_pool.tile([P, NLP, C], F32, name="kv")
            for pt in range(NLP):
                kvp = kv[:, pt, :]
                nc.vector.tensor_scalar(kvp, g_sb[:, pt, 0:C],
                                        wcorn[:, 4 * pt:4 * pt + 1], None, AL.mult)
                for dy, dx in ((0, 1), (1, 0), (1, 1)):
                    nc.vector.scalar_tensor_tensor(
                        out=kvp,
                        in0=g_sb[:, pt, 512 * dx + 256 * dy:512 * dx + 256 * dy + 256],
                        scalar=wcorn[:, 4 * pt + 2 * dy + dx:4 * pt + 2 * dy + dx + 1],
                        in1=kvp,
                        op0=AL.mult,
                        op1=AL.add,
                    )

            if stop_after == "kv":
                nc.sync.dma_start(out=out_ext[bass.ts(t, P), :], in_=kv[:, 0, :])
                continue

            # ---- logits: lg[q, 18] = <query, kv_p>; softmax over 18 ----
            lg = sm_pool.tile([P, NLP], F32, name="lg")
            scr_v = one_pool.tile([P, C], F32, name="scr_v")
            for pt in range(NLP):
                nc.vector.scalar_tensor_tensor(
                    out=scr_v[:],
                    in0=kv[:, pt, :],
                    scalar=1.0,
                    in1=q_sb[:],
                    op0=AL.mult,
                    op1=AL.mult,
                    accum_out=lg[:, pt:pt + 1],
                )
            if stop_after == "logits":
                nc.sync.dma_start(out=out_ext[bass.ts(t, P), 0:NLP], in_=lg[:])
                continue
            mx = sm_pool.tile([P, 1], F32, name="mx")
            nc.vector.tensor_reduce(mx[:], lg[:], AX.X, AL.max)
            nbias = sm_pool.tile([P, 1], F32, name="nbias")
            nc.vector.tensor_scalar(nbias[:], mx[:], -16.0, None, AL.mult)
            ex = sm_pool.tile([P, NLP], F32, name="ex")
            nc.scalar.activation(ex[:], lg[:], mybir.ActivationFunctionType.Exp,
                                 bias=nbias[:], scale=16.0)
            rs = sm_pool.tile([P, 1], F32, name="rs")
            nc.vector.tensor_reduce(rs[:], ex[:], AX.X, AL.add)
            rinv = sm_pool.tile([P, 1], F32, name="rinv")
            nc.vector.reciprocal(rinv[:], rs[:])

            if stop_after == "softmax":
                nc.sync.dma_start(out=out_ext[bass.ts(t, P), 0:NLP], in_=ex[:])
                continue

            # ---- combine: out = (sum_p ex_p * kv_p) * rinv ----
            out_sb = sm_pool.tile([P, C], F32, name="out_sb")
            nc.vector.tensor_scalar(out_sb[:], kv[:, 0, :], ex[:, 0:1],
                                    None, AL.mult)
            for pt in range(1, NLP):
                nc.vector.scalar_tensor_tensor(
                    out=out_sb[:],
                    in0=kv[:, pt, :],
                    scalar=ex[:, pt:pt + 1],
                    in1=out_sb[:],
                    op0=AL.mult,
                    op1=AL.add,
                )
            nc.vector.tensor_scalar(out_sb[:], out_sb[:], rinv[:], None, AL.mult)

            nc.sync.dma_start(out=out_ext[bass.ts(t, P), :], in_=out_sb[:])

    return nc


# ---------------- host side ----------------

def host_prep(query, value, reference_points, W_off, b_off):
    q = np.ascontiguousarray(query[0], dtype=np.float32)    # [20000, 256]
    rp = np.asarray(reference_points[0], dtype=np.float32)  # [20000, 2, 2]

    vpad = np.zeros((NLEV, HP, WP, C), np.float32)
    v = np.asarray(value, np.float32)
    vpad[:, PAD:PAD + H, PAD:PAD + W, :] = v.reshape(NLEV, H, W, C)
    # pair-interleave: vp[l, y, x] = [vpad[l, y, x], vpad[l, y+1, x]]
    vp = np.zeros((NLEV, HP, WP, 2, C), np.float32)
    vp[:, :, :, 0, :] = vpad
    vp[:, :-1, :, 1, :] = vpad[:, 1:]
    vp = np.ascontiguousarray(vp.reshape(VP_ROWS, 2 * C))

    b = np.asarray(b_off, np.float32).reshape(NLEV, NPT, 2)
    # refpx[q, (l,p,xy)] = 128*ref + b_off + 3.5  (pad shift 4, align shift -0.5)
    refpx = (np.float32(128.0) * rp[:, :, None, :] + b[None] + np.float32(3.5))
    refpx = np.ascontiguousarray(refpx.reshape(NQ, NOFF), np.float32)

    woff = np.ascontiguousarray(W_off, np.float32)          # [256, 36]
    return q, vp, refpx, woff


def pack_consts(woff, refpx_core, qp):
    """[128, CONST_FREE]: W_off | refpx(per-tile), c-chunk / tile major."""
    consts = np.empty((P, CONST_FREE), np.float32)
    consts[:, :2 * NOFF] = woff.reshape(2, P, NOFF).transpose(1, 0, 2).reshape(P, -1)
    consts[:, 2 * NOFF:] = (
        refpx_core.reshape(NT_FULL, P, NOFF).transpose(1, 0, 2).reshape(P, -1))
    return consts


def shard(q, refpx, woff):
    """pad + shard queries across cores"""
    qs, cs, qts = [], [], []
    for c in range(N_CORES):
        sl = slice(c * NQ_REAL, (c + 1) * NQ_REAL)
        qp = np.zeros((NQ_CORE, C), np.float32)
        qp[:NQ_REAL] = q[sl]
        rp = np.full((NQ_CORE, NOFF), 67.5, np.float32)  # pad rows: center coords
        rp[:NQ_REAL] = refpx[sl]
        qs.append(qp)
        cs.append(pack_consts(woff, rp, qp))
        qts.append(np.ascontiguousarray(qp.T))
    return qs, cs, qts


_NC_CACHE = {}


def kernel(query, key, value, reference_points, spatial_shapes, W_off, b_off):
    from concourse.bass_utils import run_bass_kernel_spmd

    q, vp, refpx, woff = host_prep(query, value, reference_points, W_off, b_off)
    qs, cs, qts = shard(q, refpx, woff)

    if "nc" not in _NC_CACHE:
        nc = build_nc()
        nc.finalize()
        _NC_CACHE["nc"] = nc
    nc = _NC_CACHE["nc"]

    in_maps = [
        {"q": qs[c], "qt": qts[c], "consts": cs[c], "vp": vp}
        for c in range(N_CORES)
    ]
    res = run_bass_kernel_spmd(nc, in_maps, list(range(N_CORES)))
    out = np.concatenate([res.results[c]["out"][:NQ_REAL] for c in range(N_CORES)], 0)
    return out[None].astype(np.float32)

